# revision 14
# baseline (speedup 1.0000x reference)
"""Causal multi-head attention (B=4, S=2048, D=1024, H=16) on 8 TRN2 NeuronCores.

Sharding: 4 batches x 2 head-groups (8 heads each) -> 8 cores, ReduceScatter
over head-group pairs after the output projection.

v3 pipeline (vs v2 at 504us):
  - causal mask applied ADDITIVELY on the PE (-240*I stationary x negmask
    moving, accumulated into the score PSUM before exp) instead of
    multiplicative gpsimd ops after exp.  GpSimd now only triggers the
    collectives, so an RS partner-sync stall can no longer dam up the
    mask ops and stall the whole pipeline (v2's 20us gap).
  - PE FIFO order per block: QK(kb+1) -> filler pop -> PV(kb), so the PE
    never head-of-line blocks on exp(kb), and deferred outproj work slots
    in behind the next QK without delaying ACT.
  - qT/kT/ctxT/Wo in bf16 (faster LDWEIGHTS via FWL, half the SBUF/DMA);
    PV runs only on the unmasked [qst:512] column range of diag blocks.
  - output projection partials, ReduceScatter and final output in bf16;
    RS split into 2 pieces of 256 tokens per chunk, each followed
    immediately by its out-DMA -> the end-of-kernel tail shrinks from
    ~43us (one 2MB fp32 RS + all out DMAs deferred) to ~15us.
  - softmax denominators still ride the PV matmul as a ones-column of va;
    reciprocal broadcast across partitions still via the small DRAM
    bounce (PSUM is fully budgeted: 2x4KB score ring + 4x2KB ctx ring),
    deferred 4 blocks for latency slack.
"""

import numpy as np

B, S, D = 4, 2048, 1024
H = 16
HD = D // H  # 64
G = 2  # head groups (tensor-parallel degree per batch)
HPG = H // G  # 8 heads per core
DG = D // G  # 512 dims per group
P = 128
NKT = D // P  # 8 k-tiles over d_model
NQC = S // 512  # 4 query chunks of 512
NTT = S // P  # 16 token tiles of 128
NR = DG // P  # 4 dim-tiles (head pairs) per group
NPC = 2  # RS pieces per chunk (256 tokens each)

# moving-range start per diagonal block s = kb - 4*qc (block fully masked
# below this column; the lone +1 element keeps s>=1 blocks one subblock wider)
Q_START = [0, 0, 128, 256, 256]
# additive-mask range per s (elem-column subblock + triangular subblock)
M_RANGE = [(0, 128), (0, 256), (128, 384), (256, 512), (256, 512)]

NEG = 240.0  # additive mask magnitude: exp((x - 240)/8) ~ 1e-11

_CACHE = {}


def _build_negmask():
    """negmask[s] is the [128, 512] 0/1 pattern (1 = masked) for a scoresT
    block [k_local, q_chunk_local] whose k-block index is kb = 4*qc + s,
    within the M_RANGE[s] window.  Allowed iff global k <= global q + 1."""
    masks = np.zeros((5, P, 512), dtype=np.float32)
    i = np.arange(P)[:, None]  # k local
    jj = np.arange(P)[None, :]  # q local within 128-subblock
    for s in range(5):
        for j in range(4):  # q subblock within the 512 chunk
            blk = masks[s][:, 128 * j : 128 * (j + 1)]
            if j > s:
                blk[:] = 1.0
            elif j == s:
                blk[:] = (i <= jj + 1).astype(np.float32)
            elif j == s - 1:
                blk[0, 127] = 1.0
    neg = np.zeros((5, P, 512), dtype=np.float32)
    for s in range(5):
        ms, me = M_RANGE[s]
        neg[s][:, ms:me] = 1.0 - masks[s][:, ms:me]
    return neg


def _build_bass():
    import concourse.bacc as bacc
    import concourse.mybir as mybir
    import concourse.tile as tile

    f32 = mybir.dt.float32
    f32r = mybir.dt.float32r
    bf16 = mybir.dt.bfloat16
    AF = mybir.ActivationFunctionType

    nc = bacc.Bacc("TRN2", target_bir_lowering=False, debug=False, num_devices=8)

    xT = nc.dram_tensor("xT", [D, S], f32r, kind="ExternalInput").ap()
    wq = nc.dram_tensor("wq", [D, DG], f32r, kind="ExternalInput").ap()
    wk = nc.dram_tensor("wk", [D, DG], f32r, kind="ExternalInput").ap()
    wv = nc.dram_tensor("wv", [D, DG], f32r, kind="ExternalInput").ap()
    wo = nc.dram_tensor("wo", [DG, D], bf16, kind="ExternalInput").ap()
    bo_b = nc.dram_tensor("bo_b", [P, D], f32, kind="ExternalInput").ap()
    negmask = nc.dram_tensor("negmask", [5, P, 512], bf16, kind="ExternalInput").ap()
    negI = nc.dram_tensor("negI", [P, P], bf16, kind="ExternalInput").ap()
    out_ext = nc.dram_tensor("out", [NQC, NPC, P, D], bf16, kind="ExternalOutput").ap()

    with tile.TileContext(nc) as tc:
        with (
            tc.tile_pool(name="pqk", bufs=1) as pqk,
            tc.tile_pool(name="pv", bufs=1) as pv,
            tc.tile_pool(name="pmask", bufs=1) as pmask,
            tc.tile_pool(name="pdram", bufs=1, space="DRAM") as pdram,
        ):
            # persistent SBUF tensors
            qT_sb = pqk.tile([P, NR, S], bf16)  # [dims of pair r | token]
            kT_sb = pqk.tile([P, NR, S], bf16)
            va_sb = pv.tile([P, NTT, HPG, HD + 1], bf16)  # v + ones col
            negmask_sb = pmask.tile([P, 5, 512], bf16)
            negI_sb = pmask.tile([P, P], bf16)

            # ---------------- projections ----------------
            with (
                tc.tile_pool(name="pw", bufs=3) as pw,
                tc.tile_pool(name="px", bufs=2) as px,
                tc.tile_pool(name="pp", bufs=4, space="PSUM") as pp,
            ):
                xT_r = xT.rearrange("(ko p) t -> p ko t", p=P)
                # wq + the first x tile are split per-kt so the first matmul
                # only waits on two small transfers, not the full input queue
                w_sbs = {}
                w_sb = pw.tile([P, NKT, DG], f32r, name="w_wq", tag="w")
                wq_r = wq.rearrange("(ko p) f -> p ko f", p=P)
                xtile0 = px.tile([P, NKT, 512], f32r, name="xtile", tag="x")
                for kt in range(NKT):
                    nc.sync.dma_start(w_sb[:, kt, :], wq_r[:, kt, :])
                    nc.sync.dma_start(xtile0[:, kt, :], xT_r[:, kt, 0:512])
                w_sbs["wq"] = w_sb
                for name, w in (("wk", wk), ("wv", wv)):
                    w_sb = pw.tile([P, NKT, DG], f32r, name=f"w_{name}", tag="w")
                    nc.sync.dma_start(w_sb[:], w.rearrange("(ko p) f -> p ko f", p=P))
                    w_sbs[name] = w_sb
                # not needed until attention starts; keep off the critical path
                nc.sync.dma_start(negmask_sb[:], negmask.rearrange("s p q -> p s q"))
                nc.sync.dma_start(negI_sb[:], negI)
                nc.vector.memset(va_sb[:, :, :, HD : HD + 1], 1.0)

                xtiles = {0: xtile0}
                for t in range(NQC):
                    tok = slice(512 * t, 512 * (t + 1))
                    xtile = xtiles.pop(t)
                    if t + 1 < NQC:
                        nxt = px.tile([P, NKT, 512], f32r, name="xtile", tag="x")
                        nc.sync.dma_start(
                            nxt[:], xT_r[:, :, slice(512 * (t + 1), 512 * (t + 2))]
                        )
                        xtiles[t + 1] = nxt
                    # qT / kT: out [dims(pair r), 512 tokens] -> bf16
                    for name, dst in (("wq", qT_sb), ("wk", kT_sb)):
                        w_sb = w_sbs[name]
                        for rr in range(NR):
                            ps = pp.tile([P, 512], f32, name="ps_proj", tag="ps")
                            for kt in range(NKT):
                                nc.tensor.matmul(
                                    ps[:],
                                    w_sb[:, kt, P * rr : P * (rr + 1)],
                                    xtile[:, kt, :],
                                    start=(kt == 0),
                                    stop=(kt == NKT - 1),
                                )
                            nc.vector.tensor_copy(dst[:, rr, tok], ps[:])
                    # v: out [128 tokens, 512 dims] per token tile -> bf16
                    w_sb = w_sbs["wv"]
                    for st in range(4):
                        tt = 4 * t + st
                        ps = pp.tile([P, 512], f32, name="ps_v", tag="ps")
                        for kt in range(NKT):
                            nc.tensor.matmul(
                                ps[:],
                                xtile[:, kt, 128 * st : 128 * (st + 1)],
                                w_sb[:, kt, :],
                                start=(kt == 0),
                                stop=(kt == NKT - 1),
                            )
                        nc.vector.tensor_copy(
                            va_sb[:, tt, :, 0:HD],
                            ps[:].rearrange("p (h d) -> p h d", d=HD),
                        )

            # ---------------- attention + chunked output projection ----------------
            with (
                tc.tile_pool(name="pw2", bufs=1) as pw2,
                tc.tile_pool(name="pc", bufs=2) as pc,
                tc.tile_pool(name="pe", bufs=3) as pe,
                tc.tile_pool(name="pn", bufs=2) as pn,
                tc.tile_pool(name="po_sb", bufs=3) as po_sb,
                tc.tile_pool(name="psS", bufs=2, space="PSUM") as psS,
                tc.tile_pool(name="psC", bufs=4, space="PSUM") as psC,
            ):
                wo_sb = pw2.tile([P, NR, D], bf16)
                nc.sync.dma_start(wo_sb[:], wo.rearrange("(ko p) f -> p ko f", p=P))
                bo_sb = pw2.tile([P, D], f32)
                nc.sync.dma_start(bo_sb[:], bo_b[:])

                # deferred-emission queue: one thunk popped per kb iteration,
                # placed between QK(kb+1) and PV(kb) in the PE FIFO so filler
                # never delays the next exp's input.
                pending = []
                out_dmas = []

                def pop_one():
                    if pending:
                        pending.pop(0)()

                def emit_normalize(ctx0, ctx1, ctxT, pr):
                    # denominator rows -> reciprocal -> DRAM bounce for the
                    # partition broadcast.  (custom-DVE reciprocal misbehaves
                    # on PSUM inputs; stage through SBUF first.)
                    dr = pn.tile([1, 2, 512], f32, name="dr", tag="dr")
                    nc.vector.tensor_copy(dr[:, 0, :], ctx0[HD : HD + 1, :])
                    nc.vector.tensor_copy(dr[:, 1, :], ctx1[HD : HD + 1, :])
                    rrc = pn.tile([1, 2, 512], f32, name="rrc", tag="rr")
                    nc.vector.reciprocal_approx_fast(rrc[:, 0, :], dr[:, 0, :])
                    nc.vector.reciprocal_approx_fast(rrc[:, 1, :], dr[:, 1, :])
                    srow_d = pdram.tile(
                        [1, 2, 512], f32, name="srow_d", tag="srow_d", bufs=4
                    )
                    nc.sync.dma_start(srow_d[:], rrc[:])
                    bc_sb = pn.tile([HD, 2, 512], f32, name="bc_sb", tag="bc_sb")
                    nc.sync.dma_start(
                        bc_sb[:], srow_d[0:1, :, :].to_broadcast((HD, 2, 512))
                    )

                    def thunk():
                        nc.vector.tensor_mul(
                            ctxT[0:HD, pr, :], ctx0[0:HD, :], bc_sb[:, 0, :]
                        )
                        nc.vector.tensor_mul(
                            ctxT[HD:P, pr, :], ctx1[0:HD, :], bc_sb[:, 1, :]
                        )

                    # spacer pops give the DMA round trip time to land
                    pending.extend([lambda: None] * 4)
                    pending.append(thunk)

                def emit_outproj(ctxT, qc):
                    for pc_i in range(NPC):
                        partial_d = pdram.tile(
                            [256, D], bf16, name="partial", tag="partial", bufs=4
                        )
                        rs_d = pdram.tile([P, D], bf16, name="rs", tag="rs", bufs=4)
                        for tt_h in range(2):
                            ts_ = slice(
                                256 * pc_i + 128 * tt_h, 256 * pc_i + 128 * (tt_h + 1)
                            )
                            tl_ = slice(128 * tt_h, 128 * (tt_h + 1))
                            for nch in range(2):
                                ns = slice(512 * nch, 512 * (nch + 1))

                                def thunk(ts_=ts_, tl_=tl_, ns=ns, partial_d=partial_d):
                                    ps = psS.tile([P, 512], f32, name="ps_o", tag="sc")
                                    for rr in range(NR):
                                        nc.tensor.matmul(
                                            ps[:],
                                            ctxT[:, rr, ts_],
                                            wo_sb[:, rr, ns],
                                            start=(rr == 0),
                                            stop=(rr == NR - 1),
                                        )
                                    ot = po_sb.tile([P, 512], bf16, name="ot", tag="ot")
                                    nc.vector.tensor_add(ot[:], ps[:], bo_sb[:, ns])
                                    nc.sync.dma_start(partial_d[tl_, ns], ot[:])

                                pending.append(thunk)

                        def cc_thunk(partial_d=partial_d, rs_d=rs_d):
                            nc.gpsimd.collective_compute(
                                "ReduceScatter",
                                mybir.AluOpType.add,
                                replica_groups=[[0, 1], [2, 3], [4, 5], [6, 7]],
                                ins=[partial_d.opt()],
                                outs=[rs_d.opt()],
                            )

                        def dma_thunk(rs_d=rs_d, qc=qc, pc_i=pc_i):
                            nc.sync.dma_start(out_ext[qc, pc_i], rs_d[:])

                        pending.append(cc_thunk)
                        pending.append(dma_thunk)

                for qc in range(NQC):
                    nkb = min(4 * qc + 5, NTT)
                    ctxT = pc.tile([P, NR, 512], bf16, name="ctxT", tag="ctxT")
                    for pr in range(NR):
                        ctx0 = psC.tile([HD + 1, 512], f32, name="ctx0", tag="ctx")
                        ctx1 = psC.tile([HD + 1, 512], f32, name="ctx1", tag="ctx")

                        def emit_qk(kb):
                            s = kb - 4 * qc
                            diag = s >= 0
                            qst = Q_START[s] if diag else 0
                            ks = slice(128 * kb, 128 * (kb + 1))
                            qsl = slice(512 * qc + qst, 512 * (qc + 1))
                            sc = psS.tile([P, 2, 512], f32, name="sc", tag="sc")
                            nc.tensor.matmul(
                                sc[:, 0, qst:512],
                                kT_sb[0:HD, pr, ks],
                                qT_sb[0:HD, pr, qsl],
                                start=True,
                                stop=not diag,
                            )
                            nc.tensor.matmul(
                                sc[:, 1, qst:512],
                                kT_sb[HD:P, pr, ks],
                                qT_sb[HD:P, pr, qsl],
                                start=True,
                                stop=not diag,
                            )
                            if diag:
                                ms, me = M_RANGE[s]
                                nc.tensor.matmul(
                                    sc[:, 0, ms:me],
                                    negI_sb[:],
                                    negmask_sb[:, s, ms:me],
                                    start=False,
                                    stop=True,
                                )
                                nc.tensor.matmul(
                                    sc[:, 1, ms:me],
                                    negI_sb[:],
                                    negmask_sb[:, s, ms:me],
                                    start=False,
                                    stop=True,
                                )
                            return sc, qst

                        scs = {0: emit_qk(0)}
                        for kb in range(nkb):
                            if kb + 1 < nkb:
                                scs[kb + 1] = emit_qk(kb + 1)
                            sc, qst = scs.pop(kb)
                            et = pe.tile([P, 2, 512], bf16, name="et", tag="et")
                            nc.scalar.activation(
                                et[:, :, qst:512],
                                sc[:, :, qst:512],
                                AF.Exp,
                                scale=1.0 / 8.0,
                            )
                            pop_one()
                            nc.tensor.matmul(
                                ctx0[:, qst:512],
                                va_sb[:, kb, 2 * pr, :],
                                et[:, 0, qst:512],
                                start=(kb == 0),
                                stop=(kb == nkb - 1),
                            )
                            nc.tensor.matmul(
                                ctx1[:, qst:512],
                                va_sb[:, kb, 2 * pr + 1, :],
                                et[:, 1, qst:512],
                                start=(kb == 0),
                                stop=(kb == nkb - 1),
                            )
                        emit_normalize(ctx0, ctx1, ctxT, pr)
                    emit_outproj(ctxT, qc)

                while pending:
                    pop_one()
                for t_ in out_dmas:
                    t_()

    nc.compile()
    return nc


def _in_maps(x, Wq, Wk, Wv, Wo, bo):
    import ml_dtypes

    bf16 = ml_dtypes.bfloat16
    negmask = _build_negmask().astype(bf16)
    negI = (-NEG * np.eye(P, dtype=np.float32)).astype(bf16)
    maps = []
    for c in range(8):
        b, g = c // 2, c % 2
        cols = slice(DG * g, DG * (g + 1))
        maps.append(
            {
                "xT": np.ascontiguousarray(np.asarray(x)[b].T, dtype=np.float32),
                "wq": np.ascontiguousarray(np.asarray(Wq)[:, cols], dtype=np.float32),
                "wk": np.ascontiguousarray(np.asarray(Wk)[:, cols], dtype=np.float32),
                "wv": np.ascontiguousarray(np.asarray(Wv)[:, cols], dtype=np.float32),
                "wo": np.ascontiguousarray(np.asarray(Wo)[cols, :]).astype(bf16),
                "bo_b": np.broadcast_to(
                    np.asarray(bo, dtype=np.float32) / G, (P, D)
                ).copy(),
                "negmask": negmask,
                "negI": negI,
            }
        )
    return maps


def _get_nc():
    if "nc" not in _CACHE:
        _CACHE["nc"] = _build_bass()
    return _CACHE["nc"]


def run(inputs, trace=False):
    from concourse.bass_utils import run_bass_kernel_spmd

    nc = _get_nc()
    maps = _in_maps(**inputs)
    res = run_bass_kernel_spmd(nc, maps, list(range(8)), trace=trace)
    out = np.empty((B, S, D), dtype=np.float32)
    for c in range(8):
        b, g = c // 2, c % 2
        r = np.asarray(res.results[c]["out"]).astype(np.float32)
        for qc in range(NQC):
            for pc_i in range(NPC):
                lo = 512 * qc + 256 * pc_i + 128 * g
                out[b, lo : lo + 128, :] = r[qc, pc_i]
    return out, res


def kernel(x, Wq, Wk, Wv, Wo, bo):
    out, _ = run(dict(x=x, Wq=Wq, Wk=Wk, Wv=Wv, Wo=Wo, bo=bo))
    return out


# revision 15
# speedup vs baseline: 1.1149x; 1.1149x over previous
"""Causal multi-head attention (B=4, S=2048, D=1024, H=16) on 8 TRN2 NeuronCores.

Sharding: 4 batches x 2 head-groups (8 heads each) -> 8 cores, ReduceScatter
over head-group pairs after the output projection.

v5 pipeline (vs v4 at 461us, v2 at 504us):
  - projections for x-chunks 2,3 are interleaved INTO the qc0/qc1
    attention streams as PE filler, so the scalar engine's exp stream
    starts ~60us earlier and the kernel span approaches pure-PE-busy.
  - ONE PSUM ping-pong ring (2x4KB, pools psA/psB) is shared by the QK
    score tiles, the projection accumulators and the output-projection
    accumulators: every allocation rotates pools, so consecutive QK
    tiles always land in different banks (double-buffered vs exp) and
    filler work slots in without extra PSUM.
  - causal mask applied additively on the PE: one matmul per diag block
    (-240*I stationary x h-duplicated negmask moving) accumulated into
    the score PSUM before exp; gpsimd only triggers collectives.
  - PE FIFO order per block: filler -> QK(kb+1) -> PV(kb).
  - qT/kT/ctxT/Wo bf16; PV only on the unmasked [qst:512] range.
  - outproj partials, ReduceScatter (2 pieces of 256 tokens per chunk)
    and the output in bf16, each piece's out-DMA right after its RS.
  - softmax denominators ride PV as a ones-column of va; reciprocal
    partition-broadcast via the small DRAM bounce, deferred 4 blocks.
"""

import numpy as np

B, S, D = 4, 2048, 1024
H = 16
HD = D // H  # 64
G = 2  # head groups (tensor-parallel degree per batch)
HPG = H // G  # 8 heads per core
DG = D // G  # 512 dims per group
P = 128
NKT = D // P  # 8 k-tiles over d_model
NQC = S // 512  # 4 query chunks of 512
NTT = S // P  # 16 token tiles of 128
NR = DG // P  # 4 dim-tiles (head pairs) per group
NPC = 2  # RS pieces per chunk (256 tokens each)

# moving-range start per diagonal block s = kb - 4*qc (block fully masked
# below this column; the lone +1 element keeps s>=1 blocks one subblock wider)
Q_START = [0, 0, 128, 256, 256]
# additive-mask range per s (elem-column subblock + triangular subblock)
M_RANGE = [(0, 128), (0, 256), (128, 384), (256, 512), (256, 512)]

NEG = 240.0  # additive mask magnitude: exp((x - 240)/8) ~ 1e-11

_CACHE = {}


def _build_negmask():
    """negmask[s] is the [128, 512] 0/1 pattern (1 = masked) for a scoresT
    block [k_local, q_chunk_local] whose k-block index is kb = 4*qc + s,
    within the M_RANGE[s] window.  Allowed iff global k <= global q + 1."""
    masks = np.zeros((5, P, 512), dtype=np.float32)
    i = np.arange(P)[:, None]  # k local
    jj = np.arange(P)[None, :]  # q local within 128-subblock
    for s in range(5):
        for j in range(4):  # q subblock within the 512 chunk
            blk = masks[s][:, 128 * j : 128 * (j + 1)]
            if j > s:
                blk[:] = 1.0
            elif j == s:
                blk[:] = (i <= jj + 1).astype(np.float32)
            elif j == s - 1:
                blk[0, 127] = 1.0
    neg = np.zeros((5, P, 512), dtype=np.float32)
    for s in range(5):
        ms, me = M_RANGE[s]
        neg[s][:, ms:me] = 1.0 - masks[s][:, ms:me]
    return neg


def _build_bass():
    import concourse.bacc as bacc
    import concourse.mybir as mybir
    import concourse.tile as tile

    f32 = mybir.dt.float32
    f32r = mybir.dt.float32r
    bf16 = mybir.dt.bfloat16
    AF = mybir.ActivationFunctionType

    nc = bacc.Bacc("TRN2", target_bir_lowering=False, debug=False, num_devices=8)

    xT = nc.dram_tensor("xT", [D, S], f32r, kind="ExternalInput").ap()
    wq = nc.dram_tensor("wq", [D, DG], f32r, kind="ExternalInput").ap()
    wk = nc.dram_tensor("wk", [D, DG], f32r, kind="ExternalInput").ap()
    wv = nc.dram_tensor("wv", [D, DG], f32r, kind="ExternalInput").ap()
    wo = nc.dram_tensor("wo", [DG, D], bf16, kind="ExternalInput").ap()
    bo_b = nc.dram_tensor("bo_b", [P, D], f32, kind="ExternalInput").ap()
    # negmask duplicated over the two head-halves: [5, P, 2, 512]
    negmask = nc.dram_tensor("negmask", [5, P, 2, 512], bf16, kind="ExternalInput").ap()
    negI = nc.dram_tensor("negI", [P, P], bf16, kind="ExternalInput").ap()
    out_ext = nc.dram_tensor("out", [NQC, NPC, P, D], bf16, kind="ExternalOutput").ap()

    with tile.TileContext(nc) as tc:
        with (
            tc.tile_pool(name="pqk", bufs=1) as pqk,
            tc.tile_pool(name="pv", bufs=1) as pv,
            tc.tile_pool(name="pmask", bufs=1) as pmask,
            tc.tile_pool(name="pdram", bufs=1, space="DRAM") as pdram,
            tc.tile_pool(name="pw", bufs=3) as pw,
            tc.tile_pool(name="px", bufs=2) as px,
            tc.tile_pool(name="pw2", bufs=1) as pw2,
            tc.tile_pool(name="pc", bufs=2) as pc,
            tc.tile_pool(name="pe", bufs=3) as pe,
            tc.tile_pool(name="pn", bufs=2) as pn,
            tc.tile_pool(name="po_sb", bufs=3) as po_sb,
            tc.tile_pool(name="psA", bufs=1, space="PSUM") as psA,
            tc.tile_pool(name="psB", bufs=1, space="PSUM") as psB,
            tc.tile_pool(name="psC", bufs=4, space="PSUM") as psC,
        ):
            # persistent SBUF tensors
            qT_sb = pqk.tile([P, NR, S], bf16)  # [dims of pair r | token]
            kT_sb = pqk.tile([P, NR, S], bf16)
            va_sb = pv.tile([P, NTT, HPG, HD + 1], bf16)  # v + ones col
            negmask_sb = pmask.tile([P, 5, 2, 512], bf16)
            negI_sb = pmask.tile([P, P], bf16)

            # ---------------- input DMAs ----------------
            xT_r = xT.rearrange("(ko p) t -> p ko t", p=P)
            # wq + the first x tile are split per-kt so the first matmul
            # only waits on two small transfers, not the full input queue
            w_sbs = {}
            w_sb = pw.tile([P, NKT, DG], f32r, name="w_wq", tag="w")
            wq_r = wq.rearrange("(ko p) f -> p ko f", p=P)
            xtile0 = px.tile([P, NKT, 512], f32r, name="xtile", tag="x")
            for kt in range(NKT):
                nc.sync.dma_start(w_sb[:, kt, :], wq_r[:, kt, :])
                nc.sync.dma_start(xtile0[:, kt, :], xT_r[:, kt, 0:512])
            w_sbs["wq"] = w_sb
            for name, w in (("wk", wk), ("wv", wv)):
                w_sb = pw.tile([P, NKT, DG], f32r, name=f"w_{name}", tag="w")
                nc.sync.dma_start(w_sb[:], w.rearrange("(ko p) f -> p ko f", p=P))
                w_sbs[name] = w_sb
            nc.sync.dma_start(negmask_sb[:], negmask.rearrange("s p h q -> p s h q"))
            nc.sync.dma_start(negI_sb[:], negI)
            nc.vector.memset(va_sb[:, :, :, HD : HD + 1], 1.0)

            xtiles = {}

            def emit_xdma(t):
                nxt = px.tile([P, NKT, 512], f32r, name="xtile", tag="x")
                nc.sync.dma_start(nxt[:], xT_r[:, :, slice(512 * t, 512 * (t + 1))])
                xtiles[t] = nxt

            xtiles[0] = xtile0
            emit_xdma(1)

            wo_sb = pw2.tile([P, NR, D], bf16)
            nc.sync.dma_start(wo_sb[:], wo.rearrange("(ko p) f -> p ko f", p=P))
            bo_sb = pw2.tile([P, D], f32)
            nc.sync.dma_start(bo_sb[:], bo_b[:])

            # one PSUM ping-pong ring shared by QK scores / proj / outproj:
            # every allocation alternates pools psA/psB (1 slot of 4KB each)
            sc_i = [0]

            def sc_alloc(name):
                pool = (psA, psB)[sc_i[0] % 2]
                sc_i[0] += 1
                return pool.tile([P, 2, 512], f32, name=name, tag="sc")

            # ---------------- projection groups ----------------
            def pg_qk(dst, wname, rr, t):
                def thunk(dst=dst, wname=wname, rr=rr, t=t):
                    w_sb = w_sbs[wname]
                    xtile = xtiles[t]
                    ps = sc_alloc("ps_proj")
                    for kt in range(NKT):
                        nc.tensor.matmul(
                            ps[:, 0, :],
                            w_sb[:, kt, P * rr : P * (rr + 1)],
                            xtile[:, kt, :],
                            start=(kt == 0),
                            stop=(kt == NKT - 1),
                        )
                    nc.vector.tensor_copy(
                        dst[:, rr, slice(512 * t, 512 * (t + 1))], ps[:, 0, :]
                    )

                return thunk

            def pg_v(t, st):
                def thunk(t=t, st=st):
                    w_sb = w_sbs["wv"]
                    xtile = xtiles[t]
                    ps = sc_alloc("ps_v")
                    for kt in range(NKT):
                        nc.tensor.matmul(
                            ps[:, 0, :],
                            xtile[:, kt, 128 * st : 128 * (st + 1)],
                            w_sb[:, kt, :],
                            start=(kt == 0),
                            stop=(kt == NKT - 1),
                        )
                    nc.vector.tensor_copy(
                        va_sb[:, 4 * t + st, :, 0:HD],
                        ps[:, 0, :].rearrange("p (h d) -> p h d", d=HD),
                    )

                return thunk

            def chunk_groups(t):
                gs = []
                for rr in range(NR):
                    gs.append(pg_qk(kT_sb, "wk", rr, t))
                for st in range(4):
                    gs.append(pg_v(t, st))
                for rr in range(NR):
                    gs.append(pg_qk(qT_sb, "wq", rr, t))
                return gs

            # chunks 0,1 inline (before any attention); 2,3 become filler
            for t in (0, 1):
                for g_ in chunk_groups(t):
                    g_()
            emit_xdma(2)
            proj_fill = chunk_groups(2)
            proj_fill.append(lambda: emit_xdma(3))
            proj_fill += chunk_groups(3)

            # deferred-emission queue: popped once per block as PE filler
            pending = []

            def filler():
                if pending:
                    pending.pop(0)()
                if proj_fill:
                    proj_fill.pop(0)()

            def emit_normalize(ctx0, ctx1, ctxT, pr):
                # denominator rows -> reciprocal -> DRAM bounce for the
                # partition broadcast.  (custom-DVE reciprocal misbehaves
                # on PSUM inputs; stage through SBUF first.)
                dr = pn.tile([1, 2, 512], f32, name="dr", tag="dr")
                nc.vector.tensor_copy(dr[:, 0, :], ctx0[HD : HD + 1, :])
                nc.vector.tensor_copy(dr[:, 1, :], ctx1[HD : HD + 1, :])
                rrc = pn.tile([1, 2, 512], f32, name="rrc", tag="rr")
                nc.vector.reciprocal_approx_fast(rrc[:, 0, :], dr[:, 0, :])
                nc.vector.reciprocal_approx_fast(rrc[:, 1, :], dr[:, 1, :])
                srow_d = pdram.tile(
                    [1, 2, 512], f32, name="srow_d", tag="srow_d", bufs=4
                )
                nc.sync.dma_start(srow_d[:], rrc[:])
                bc_sb = pn.tile([HD, 2, 512], f32, name="bc_sb", tag="bc_sb")
                nc.sync.dma_start(
                    bc_sb[:], srow_d[0:1, :, :].to_broadcast((HD, 2, 512))
                )

                def thunk(ctx0=ctx0, ctx1=ctx1, ctxT=ctxT, pr=pr, bc_sb=bc_sb):
                    nc.vector.tensor_mul(
                        ctxT[0:HD, pr, :], ctx0[0:HD, :], bc_sb[:, 0, :]
                    )
                    nc.vector.tensor_mul(
                        ctxT[HD:P, pr, :], ctx1[0:HD, :], bc_sb[:, 1, :]
                    )

                # spacer pops give the DMA round trip time to land
                pending.extend([lambda: None] * 4)
                pending.append(thunk)

            def emit_outproj(ctxT, qc):
                for pc_i in range(NPC):
                    partial_d = pdram.tile(
                        [256, D], bf16, name="partial", tag="partial", bufs=4
                    )
                    rs_d = pdram.tile([P, D], bf16, name="rs", tag="rs", bufs=4)
                    for tt_h in range(2):
                        ts_ = slice(
                            256 * pc_i + 128 * tt_h, 256 * pc_i + 128 * (tt_h + 1)
                        )
                        tl_ = slice(128 * tt_h, 128 * (tt_h + 1))
                        for nch in range(2):
                            ns = slice(512 * nch, 512 * (nch + 1))

                            def thunk(
                                ts_=ts_, tl_=tl_, ns=ns, ctxT=ctxT, partial_d=partial_d
                            ):
                                ps = sc_alloc("ps_o")
                                for rr in range(NR):
                                    nc.tensor.matmul(
                                        ps[:, 0, :],
                                        ctxT[:, rr, ts_],
                                        wo_sb[:, rr, ns],
                                        start=(rr == 0),
                                        stop=(rr == NR - 1),
                                    )
                                ot = po_sb.tile([P, 512], bf16, name="ot", tag="ot")
                                nc.vector.tensor_add(ot[:], ps[:, 0, :], bo_sb[:, ns])
                                nc.sync.dma_start(partial_d[tl_, ns], ot[:])

                            pending.append(thunk)

                    def cc_thunk(partial_d=partial_d, rs_d=rs_d):
                        nc.gpsimd.collective_compute(
                            "ReduceScatter",
                            mybir.AluOpType.add,
                            replica_groups=[[0, 1], [2, 3], [4, 5], [6, 7]],
                            ins=[partial_d.opt()],
                            outs=[rs_d.opt()],
                        )

                    def dma_thunk(rs_d=rs_d, qc=qc, pc_i=pc_i):
                        nc.sync.dma_start(out_ext[qc, pc_i], rs_d[:])

                    pending.append(cc_thunk)
                    pending.append(dma_thunk)

            # ---------------- attention ----------------
            for qc in range(NQC):
                nkb = min(4 * qc + 5, NTT)
                ctxT = pc.tile([P, NR, 512], bf16, name="ctxT", tag="ctxT")
                for pr in range(NR):
                    ctx0 = psC.tile([HD + 1, 512], f32, name="ctx0", tag="ctx")
                    ctx1 = psC.tile([HD + 1, 512], f32, name="ctx1", tag="ctx")

                    def emit_qk_exp(kb, pr=pr, qc=qc):
                        s = kb - 4 * qc
                        diag = s >= 0
                        qst = Q_START[s] if diag else 0
                        ks = slice(128 * kb, 128 * (kb + 1))
                        qsl = slice(512 * qc + qst, 512 * (qc + 1))
                        sc = sc_alloc("sc")
                        nc.tensor.matmul(
                            sc[:, 0, qst:512],
                            kT_sb[0:HD, pr, ks],
                            qT_sb[0:HD, pr, qsl],
                            start=True,
                            stop=not diag,
                        )
                        nc.tensor.matmul(
                            sc[:, 1, qst:512],
                            kT_sb[HD:P, pr, ks],
                            qT_sb[HD:P, pr, qsl],
                            start=True,
                            stop=not diag,
                        )
                        if diag:
                            ms, me = M_RANGE[s]
                            nc.tensor.matmul(
                                sc[:, :, ms:me],
                                negI_sb[:],
                                negmask_sb[:, s, :, ms:me],
                                start=False,
                                stop=True,
                                skip_group_check=True,
                            )
                        et = pe.tile([P, 2, 512], bf16, name="et", tag="et")
                        nc.scalar.activation(
                            et[:, :, qst:512],
                            sc[:, :, qst:512],
                            AF.Exp,
                            scale=1.0 / 8.0,
                        )
                        return et, qst

                    ets = {0: emit_qk_exp(0)}
                    for kb in range(nkb):
                        filler()
                        if kb + 1 < nkb:
                            ets[kb + 1] = emit_qk_exp(kb + 1)
                        et, qst = ets.pop(kb)
                        nc.tensor.matmul(
                            ctx0[:, qst:512],
                            va_sb[:, kb, 2 * pr, :],
                            et[:, 0, qst:512],
                            start=(kb == 0),
                            stop=(kb == nkb - 1),
                        )
                        nc.tensor.matmul(
                            ctx1[:, qst:512],
                            va_sb[:, kb, 2 * pr + 1, :],
                            et[:, 1, qst:512],
                            start=(kb == 0),
                            stop=(kb == nkb - 1),
                        )
                    emit_normalize(ctx0, ctx1, ctxT, pr)
                emit_outproj(ctxT, qc)

            while pending or proj_fill:
                filler()

    nc.compile()
    return nc


def _in_maps(x, Wq, Wk, Wv, Wo, bo):
    import ml_dtypes

    bf16 = ml_dtypes.bfloat16
    neg = _build_negmask()  # [5, P, 512]
    negmask2 = np.repeat(neg[:, :, None, :], 2, axis=2).astype(bf16)  # [5,P,2,512]
    negI = (-NEG * np.eye(P, dtype=np.float32)).astype(bf16)
    maps = []
    for c in range(8):
        b, g = c // 2, c % 2
        cols = slice(DG * g, DG * (g + 1))
        maps.append(
            {
                "xT": np.ascontiguousarray(np.asarray(x)[b].T, dtype=np.float32),
                "wq": np.ascontiguousarray(np.asarray(Wq)[:, cols], dtype=np.float32),
                "wk": np.ascontiguousarray(np.asarray(Wk)[:, cols], dtype=np.float32),
                "wv": np.ascontiguousarray(np.asarray(Wv)[:, cols], dtype=np.float32),
                "wo": np.ascontiguousarray(np.asarray(Wo)[cols, :]).astype(bf16),
                "bo_b": np.broadcast_to(
                    np.asarray(bo, dtype=np.float32) / G, (P, D)
                ).copy(),
                "negmask": negmask2,
                "negI": negI,
            }
        )
    return maps


def _get_nc():
    if "nc" not in _CACHE:
        _CACHE["nc"] = _build_bass()
    return _CACHE["nc"]


def run(inputs, trace=False):
    from concourse.bass_utils import run_bass_kernel_spmd

    nc = _get_nc()
    maps = _in_maps(**inputs)
    res = run_bass_kernel_spmd(nc, maps, list(range(8)), trace=trace)
    out = np.empty((B, S, D), dtype=np.float32)
    for c in range(8):
        b, g = c // 2, c % 2
        r = np.asarray(res.results[c]["out"]).astype(np.float32)
        for qc in range(NQC):
            for pc_i in range(NPC):
                lo = 512 * qc + 256 * pc_i + 128 * g
                out[b, lo : lo + 128, :] = r[qc, pc_i]
    return out, res


def kernel(x, Wq, Wk, Wv, Wo, bo):
    out, _ = run(dict(x=x, Wq=Wq, Wk=Wk, Wv=Wv, Wo=Wo, bo=bo))
    return out
